# revision 18
# baseline (speedup 1.0000x reference)
"""Trainium2 Bass kernel for Mistral-style sliding-window GQA attention.

Problem (hardcoded shapes):
  hidden_states [2048, 4096] f32, Wq [4096, 4096], Wk/Wv [4096, 1024],
  Wo [4096, 4096], cu_seqlens [3] int32. 32 Q heads / 8 KV heads,
  head_dim 128, sliding window 512, rope theta 10000.

Sharding: tensor-parallel over heads across 8 cores. Core c owns Q heads
[4c, 4c+4) and KV head c. Wq/Wk/Wv are column-sharded, Wo row-sharded;
each core emits a partial [2048, 4096] fp16 output which the host sums.

Speed strategy: all four projections (Q/K/V/O) run as fp8e4m3 DoubleRow
matmuls (PE processes 2 contraction k-tiles per instruction at 0.5
cycles/row = 4x bf16 throughput). Plain fp8 is far too lossy here
(concentrated attention; rel err 6%), so every projection uses a 3-term
error-compensated decomposition
    A@B ~= A8@B8 + dA8@B8 + A8@dB8,   dX8 = fp8(X - fp8(X))
which costs 1.5 DR instructions per k-tile pair (0.75x bf16 cycles) and
restores ~bf16 accuracy. For Q/K/V both corrections (d_hidden, d_weight)
are precomputed on the host; for the O projection the activation
residual dat8 is computed on-device (one DVE subtract per head tile).
Attention proper (scores, exp, mask, PV) stays in fp16 - higher
precision than bf16 at identical modeled cost.
"""

import numpy as np
import ml_dtypes

import concourse.bass as bass
import concourse.tile as tile
from concourse import bacc, mybir
from concourse import bass_utils

# ---- problem constants -------------------------------------------------
T = 2048
HID = 4096
NUM_HEADS = 32
NUM_KV_HEADS = 8
D = 128  # head dim
WINDOW = 512
ROPE_THETA = 10000.0
N_CORES = 8
HPC = NUM_HEADS // N_CORES  # 4 q heads per core
QD = HPC * D  # 512 q-proj cols per core

NT = T // 128  # 16 token tiles
NKT = HID // 128  # 32 hidden k-tiles
NSTRIP = T // 512  # 4 token strips of 512
NOUT = HID // 512  # 8 output column slices

F32 = mybir.dt.float32
F16 = mybir.dt.float16
F8E4 = mybir.dt.float8e4
F8E5 = mybir.dt.float8e5
DR = mybir.MatmulPerfMode.DoubleRow
E4 = ml_dtypes.float8_e4m3fn
E5 = ml_dtypes.float8_e5m2
SCALE = 1.0 / np.sqrt(D)

_cache = {}


def _split8(x, hi_dt=E4, lo_dt=E4):
    """x ~= hi + lo, both fp8."""
    x64 = np.asarray(x, np.float64)
    hi = x64.astype(np.float32).astype(hi_dt)
    lo = (x64 - hi.astype(np.float64)).astype(np.float32).astype(lo_dt)
    return hi, lo


def _host_prep(cu_seqlens):
    """Everything derived from cu_seqlens: positions, rope tables,
    per-tile job list and mask tiles (ST layout [k, q], head-pair
    duplicated to [128, 256])."""
    cu = np.asarray(cu_seqlens, dtype=np.int64)
    tok = np.arange(T)
    seg = np.searchsorted(cu[1:], tok, side="right")
    pos = tok - cu[np.minimum(seg, len(cu) - 1)]

    same = seg[:, None] == seg[None, :]
    causal = pos[None, :] <= pos[:, None]
    win = pos[None, :] >= pos[:, None] - (WINDOW - 1)
    allowed = same & causal & win  # [q, k]

    jobs = []  # jobs[i] = [(j, mask_id | None), ...]
    masks = []
    mask_index = {}
    for i in range(NT):
        row = []
        for j in range(NT):
            blk = allowed[128 * i : 128 * (i + 1), 128 * j : 128 * (j + 1)]
            if not blk.any():
                continue
            if blk.all():
                row.append((j, None))
            else:
                key = blk.tobytes()
                if key not in mask_index:
                    mask_index[key] = len(masks)
                    masks.append(blk.T.astype(np.float32))  # ST layout
                row.append((j, mask_index[key]))
        jobs.append(row)
    if not masks:
        masks.append(np.ones((128, 128), np.float32))
    m = np.stack(masks)
    masks_np = np.concatenate([m, m, m, m], axis=2).astype(np.float16)

    inv = 1.0 / (ROPE_THETA ** (np.arange(0, D, 2, dtype=np.float64) / D))
    freqs = pos[:, None].astype(np.float64) * inv[None, :]  # [T, 64]
    emb = np.concatenate([freqs, freqs], axis=1)  # [T, 128]
    cos_t = emb.T.astype(np.float64)
    cos_t = np.cos(cos_t).astype(np.float16).copy()  # [128, T]
    sin_t = np.sin(np.concatenate([freqs, freqs], axis=1).T)
    # sign-folded: rope(x)[d] = x[d]*cos[d] + x[(d+64)%128] * sin_s[d]
    sin_s = np.concatenate([-sin_t[:64], sin_t[64:]], axis=0)
    sin_s = sin_s.astype(np.float16).copy()
    ident = np.eye(128, dtype=np.float16)

    return jobs, masks_np, cos_t, sin_s, ident


def _build(jobs, n_masks):
    """Trace the Bass/Tile program (identical on all cores)."""
    nc = bacc.Bacc("TRN2", target_bir_lowering=False, debug=False,
                   num_devices=N_CORES)

    # DRAM I/O (per-core shapes)
    # sht[s, gg] = [128, 8192] fp8 = two groups of [4 ht8 k-tiles | 4
    # dht8 k-tiles] of 512 tokens each (dht8 = fp8 residual of hidden^T).
    sht_d = nc.dram_tensor("sht", [NSTRIP, NKT // 8, 128, 8192], F8E4,
                           kind="ExternalInput").ap()
    # packed projection weights: per k-chunk g (4 k-tiles = 512 rows),
    # the 6 lhsT arrays [q0..q3, k, v] side by side -> [128, 6*512].
    wpk4_d = nc.dram_tensor("wpk4", [8, 128, 3072], F8E4,
                            kind="ExternalInput").ap()
    wpk5_d = nc.dram_tensor("wpk5", [8, 128, 3072], F8E5,
                            kind="ExternalInput").ap()
    # wo8 layout [128, ns(8) x hp(2) x m(2) x 512]
    wo8_d = nc.dram_tensor("wo8", [2, 128, 8192], F8E4,
                           kind="ExternalInput").ap()
    dwo8_d = nc.dram_tensor("dwo8", [2, 128, 8192], F8E5,
                            kind="ExternalInput").ap()
    cos_d = nc.dram_tensor("cos_t", [128, T], F16, kind="ExternalInput").ap()
    sin_d = nc.dram_tensor("sin_s", [128, T], F16, kind="ExternalInput").ap()
    ident_d = nc.dram_tensor("ident", [128, 128], F16,
                             kind="ExternalInput").ap()
    masks_d = nc.dram_tensor("masks", [128, n_masks * 512], F16,
                             kind="ExternalInput").ap()
    out_d = nc.dram_tensor("out", [T, HID], F16, kind="ExternalOutput").ap()

    with tile.TileContext(nc) as tc:
        with tc.tile_pool(name="persist", bufs=1) as pp:
            # resident weights / tables
            wpk4_sb = pp.tile([128, 8 * 3072], F8E4, name="wpk4_sb")
            wpk5_sb = pp.tile([128, 8 * 3072], F8E5, name="wpk5_sb")
            wo_sb = pp.tile([128, 2 * 8192], F8E4, name="wo_sb")
            dwo_sb = pp.tile([128, 2 * 8192], F8E5, name="dwo_sb")
            cos_sb = pp.tile([128, T], F16, name="cos_sb")
            sin_sb = pp.tile([128, T], F16, name="sin_sb")
            ident_sb = pp.tile([128, 128], F16, name="ident_sb")
            mask_sb = pp.tile([128, n_masks * 512], F16, name="mask_sb")
            # activations produced by phase 1, consumed by phase 2
            # qt pairs: [128, 2*T]; cols [256*i + 128*m : +128] = head
            # (2*hp + m), token tile i.
            qt_sb = [pp.tile([128, 2 * T], F16, name=f"qtp{hp}")
                     for hp in range(2)]
            kt_sb = pp.tile([128, T], F16, name="kt_sb")
            vaug_sb = [pp.tile([128, D + 1], F16, name=f"vaug{t}")
                       for t in range(NT)]

            qt_4d = [q.rearrange("p (i m c) -> p i m c", m=2, c=128)
                     for q in qt_sb]
            # [128, ns, hp, m, 512] views of the O-proj weights
            wo_5d = wo_sb.rearrange("p (ns hp m n) -> p ns hp m n",
                                    ns=8, hp=2, m=2)
            dwo_5d = dwo_sb.rearrange("p (ns hp m n) -> p ns hp m n",
                                      ns=8, hp=2, m=2)

            for t in range(NT):
                nc.vector.memset(vaug_sb[t][:, D : D + 1], 1.0)

            # ---------------- phase 1: projections + RoPE ----------------
            with (
                tc.tile_pool(name="ht_pool", bufs=6) as htp,
                tc.tile_pool(name="rope_tmp", bufs=4) as rtp,
                tc.tile_pool(name="proj_psum", bufs=6, space="PSUM") as ppp,
                tc.tile_pool(name="util_psum", bufs=2, space="PSUM") as upp,
            ):
                def rope(s, h, src):
                    """src: fp32 PSUM [128, 512] pre-rope projection."""
                    ssl = bass.ts(s, 512)
                    if h < HPC:
                        dst = qt_4d[h // 2][:, 4 * s : 4 * s + 4, h % 2, :]
                    else:
                        dst = kt_sb[:, ssl]
                    raw = rtp.tile([128, 512], F16, tag="raw",
                                   name=f"raw{s}_{h}")
                    nc.scalar.copy(raw[:], src[:])
                    t1 = rtp.tile([128, 512], F16, tag="t1",
                                  name=f"t1_{s}_{h}")
                    nc.gpsimd.tensor_mul(t1[:], raw[:], cos_sb[:, ssl])
                    # rotate_half: walrus requires TT operands to share a
                    # start partition, so swap halves via TS copies first
                    # (partition-shifted copies are legal; signs live in sin_s)
                    sw = rtp.tile([128, 512], F16, tag="sw",
                                  name=f"sw{s}_{h}")
                    nc.vector.tensor_scalar_mul(sw[0:64, :],
                                                raw[64:128, :], 1.0)
                    nc.vector.tensor_scalar_mul(sw[64:128, :],
                                                raw[0:64, :], 1.0)
                    t2 = rtp.tile([128, 512], F16, tag="t2",
                                  name=f"t2_{s}_{h}")
                    nc.vector.tensor_mul(t2[:], sw[:], sin_sb[:, ssl])
                    if h < HPC:
                        t1v = t1.rearrange("p (i c) -> p i c", c=128)
                        t2v = t2.rearrange("p (i c) -> p i c", c=128)
                    else:
                        t1v, t2v = t1[:], t2[:]
                    nc.vector.tensor_add(dst, t1v, t2v)

                def v_pipeline(s, ps_v):
                    """ps_v: vT strip PSUM -> 4 v_aug tiles [k, dim]."""
                    vts = rtp.tile([128, 512], F16, tag="vts", name=f"vts{s}")
                    nc.vector.tensor_copy(vts[:], ps_v[:])
                    vtp = upp.tile([128, 512], F16, tag="util", name=f"vtp{s}")
                    for tt in range(4):
                        tsl = bass.ts(tt, 128)
                        nc.tensor.transpose(vtp[:, tsl], vts[:, tsl],
                                            ident_sb[:])
                        nc.vector.tensor_copy(vaug_sb[4 * s + tt][:, 0:D],
                                              vtp[:, tsl])

                # packed weight views: [128, g(8), a(6), j(4 k-tiles), 128]
                w4_v = wpk4_sb.rearrange("p (g a j m) -> p g a j m",
                                         g=8, a=6, j=4)
                w5_v = wpk5_sb.rearrange("p (g a j m) -> p g a j m",
                                         g=8, a=6, j=4)

                def wpair(view, a, kp):
                    """lhsT [128, 2, 128] for array a, k-tile pair kp."""
                    return view[:, kp // 2, a, 2 * (kp % 2) : 2 * (kp % 2) + 2, :]

                def proj_round(s, heads, preamble=None, postamble=None):
                    """One k-loop computing projections `heads` (0..3 = q,
                    4 = k, 5 = v) for strip s into len(heads) PSUM banks.
                    3-term fp8 DoubleRow per k-tile pair."""
                    ps = [ppp.tile([128, 512], F32, tag="proj",
                                   name=f"ps{s}_{h}") for h in heads]
                    npair = NKT // 2  # 16 k-tile pairs
                    t3_pend = []  # deferred (kp, h8) term-3 operands
                    for gg in range(NKT // 8):
                        if preamble is not None:
                            preamble(gg)
                        # one DMA carries 8 hidden k-tiles + their residuals
                        ht_t = htp.tile([128, 8192], F8E4, tag="ht",
                                        name=f"ht{s}_{gg}_{heads[0]}")
                        if s == 0 and gg == 0 and preamble is not None:
                            nc.sync.dma_start(ht_t[:, 0:4096],
                                              sht_d[s, gg][:, 0:4096])
                            nc.sync.dma_start(ht_t[:, 4096:8192],
                                              sht_d[s, gg][:, 4096:8192])
                        else:
                            nc.sync.dma_start(ht_t[:], sht_d[s, gg])
                        if postamble is not None:
                            postamble(gg)
                        ht_4d = ht_t.rearrange("p (cc c n) -> p cc c n",
                                               cc=2, n=512)
                        # chunk 0 of strip 0: all t1 first (they only need
                        # the first half of the split ht DMA)
                        split0 = heads[0] == 0 and s == 0 and gg == 0 \
                            and preamble is not None
                        for t2_pass in range(2 if split0 else 1):
                            for p in range(4):
                                kp = 4 * gg + p  # k-tile pair index
                                cc, pl = p // 2, p % 2
                                h8 = ht_4d[:, cc, 2 * pl : 2 * pl + 2, :]
                                dh8 = ht_4d[:, cc, 4 + 2 * pl : 6 + 2 * pl, :]
                                for ps_t, h in zip(ps, heads):
                                    if not (split0 and t2_pass == 1):
                                        nc.tensor.matmul(
                                            ps_t[:], wpair(w4_v, h, kp), h8,
                                            start=(kp == 0), stop=False,
                                            perf_mode=DR)
                                    if not (split0 and t2_pass == 0):
                                        nc.tensor.matmul(
                                            ps_t[:], wpair(w4_v, h, kp), dh8,
                                            start=False, stop=False,
                                            perf_mode=DR)
                        # term 3 (e5m2 weight corrections) deferred two
                        # chunks so those DMAs never gate the PE
                        for p in range(4):
                            kp = 4 * gg + p
                            cc, pl = p // 2, p % 2
                            t3_pend.append(
                                (kp, ht_4d[:, cc, 2 * pl : 2 * pl + 2, :]))
                        while len(t3_pend) > 8:
                            kp, h8 = t3_pend.pop(0)
                            for ps_t, h in zip(ps, heads):
                                nc.tensor.matmul(ps_t[:], wpair(w5_v, h, kp),
                                                 h8, start=False, stop=False,
                                                 perf_mode=DR)
                    while t3_pend:
                        kp, h8 = t3_pend.pop(0)
                        last = kp == npair - 1
                        for ps_t, h in zip(ps, heads):
                            nc.tensor.matmul(ps_t[:], wpair(w5_v, h, kp),
                                             h8, start=False, stop=last,
                                             perf_mode=DR)
                    return ps

                def strip0_preamble(gg):
                    if gg == 0:
                        # split so the very first matmuls start sooner
                        nc.sync.dma_start(wpk4_sb[:, 0:1536],
                                          wpk4_d[0][:, 0:1536])
                        nc.sync.dma_start(wpk4_sb[:, 1536:3072],
                                          wpk4_d[0][:, 1536:3072])
                    else:
                        for g in (2 * gg, 2 * gg + 1):
                            nc.sync.dma_start(wpk4_sb[:, bass.ts(g, 3072)],
                                              wpk4_d[g])

                def strip0_postamble(gg):
                    if gg == 0:
                        nc.sync.dma_start(wpk4_sb[:, bass.ts(1, 3072)],
                                          wpk4_d[1])
                    for g in (2 * gg, 2 * gg + 1):
                        nc.sync.dma_start(wpk5_sb[:, bass.ts(g, 3072)],
                                          wpk5_d[g])
                    if gg == 0:
                        nc.sync.dma_start(ident_sb[:], ident_d)
                    elif gg == 1:
                        nc.sync.dma_start(cos_sb[:], cos_d)
                    elif gg == 2:
                        nc.sync.dma_start(sin_sb[:], sin_d)
                    else:
                        nc.sync.dma_start(mask_sb[:], masks_d)

                for s in range(NSTRIP - 1):
                    ps = proj_round(s, [0, 1, 2, 3, 4, 5],
                                    preamble=strip0_preamble if s == 0 else None,
                                    postamble=strip0_postamble if s == 0 else None)
                    if s >= 1:
                        # wo is only needed in phase 2; trickle it in
                        nc.sync.dma_start(
                            wo_sb[:, bass.ts(s - 1, 8192)], wo8_d[s - 1])
                        nc.sync.dma_start(
                            dwo_sb[:, bass.ts(s - 1, 8192)], dwo8_d[s - 1])
                    v_pipeline(s, ps[5])
                    for h in range(HPC + 1):
                        rope(s, h, ps[h])

                # Last strip in two 3-bank rounds (ht re-streamed): round A's
                # banks drain during round B's matmuls, so phase 2's PSUM
                # pools don't stall on the phase-1 epilogue.
                s = NSTRIP - 1
                ps_a = proj_round(s, [0, 1, 4])
                for h in (0, 1, 4):
                    rope(s, h, ps_a[(0, 1, 4).index(h)])
                ps_b = proj_round(s, [5, 2, 3])
                v_pipeline(s, ps_b[0])
                for h in (2, 3):
                    rope(s, h, ps_b[(5, 2, 3).index(h)])

            # ---------------- phase 2: attention + out proj --------------
            with (
                tc.tile_pool(name="attn_sbuf", bufs=8) as asp,
                tc.tile_pool(name="attn_small", bufs=4) as asmall,
                tc.tile_pool(name="at_pool", bufs=4) as atp_pool,
                tc.tile_pool(name="score_psum", bufs=3, space="PSUM") as spp,
                tc.tile_pool(name="oaug_psum", bufs=2, space="PSUM") as opp,
                tc.tile_pool(name="oproj_psum", bufs=3, space="PSUM") as prp,
            ):

                def oproj(i, atbuf):
                    """atbuf: [128, 1024] fp8; cols [128h] = at8 head h,
                    [512+128h] = dat8 head h."""
                    isl = bass.ts(i, 128)
                    at4 = atbuf.rearrange("p (x m) -> p x m", m=128)
                    po_sb = asp.tile([128, HID], F16, tag="posb", bufs=2,
                                     name=f"posb{i}")
                    for ns in range(NOUT):
                        po = prp.tile([128, 512], F32, tag="oproj",
                                      name=f"po{i}_{ns}")
                        for hp in range(2):
                            amain = at4[:, 2 * hp : 2 * hp + 2, :]
                            adat = at4[:, 4 + 2 * hp : 6 + 2 * hp, :]
                            wpair = wo_5d[:, ns, hp]
                            dwpair = dwo_5d[:, ns, hp]
                            nc.tensor.matmul(po[:], amain, wpair,
                                             start=(hp == 0), stop=False,
                                             perf_mode=DR)
                            nc.tensor.matmul(po[:], adat, wpair,
                                             start=False, stop=False,
                                             perf_mode=DR)
                            nc.tensor.matmul(po[:], amain, dwpair,
                                             start=False, stop=(hp == 1),
                                             perf_mode=DR)
                        osl = po_sb[:, bass.ts(ns, 512)]
                        if ns % 2 == 0:
                            nc.vector.tensor_copy(osl, po[:])
                        else:
                            nc.scalar.copy(osl, po[:])
                        if ns == 3:
                            nc.sync.dma_start(out_d[isl, 0:2048],
                                              po_sb[:, 0:2048])
                    nc.sync.dma_start(out_d[isl, 2048:4096],
                                      po_sb[:, 2048:4096])

                def at_split(i, a_n_tiles):
                    """PE-transpose the 4 normalized head tiles of token
                    tile i and split each into at8 + dat8 fp8 halves.
                    Deferred one iteration so the PE queue never waits on
                    the ACT normalize chain."""
                    atbuf = atp_pool.tile([128, 1024], F8E4, tag="at",
                                          name=f"atb{i}")
                    for h in range(4):
                        at_p = spp.tile([128, 128], F16, tag="score",
                                        name=f"atp{i}_{h}")
                        nc.tensor.transpose(at_p[:], a_n_tiles[h][:],
                                            ident_sb[:])
                        a8sl = atbuf[:, bass.ds(128 * h, 128)]
                        nc.vector.tensor_copy(a8sl, at_p[:])
                        nc.vector.tensor_sub(
                            atbuf[:, bass.ds(512 + 128 * h, 128)],
                            at_p[:], a8sl)
                    return atbuf

                def scores_block(i):
                    """Scores + exp + mask for all 4 heads of tile i; one
                    [128,512] bank per j-tile (head h at cols 128h)."""
                    se_list = []
                    for n, (j, mid) in enumerate(jobs[i]):
                        ps_s = spp.tile([128, 512], F32, tag="score",
                                        name=f"pss{i}_{n}")
                        for hp in range(2):
                            nc.tensor.matmul(
                                ps_s[:, bass.ts(hp, 256)],
                                kt_sb[:, bass.ts(j, 128)],
                                qt_sb[hp][:, bass.ts(i, 256)],
                                start=True, stop=True)
                        se = asp.tile([128, 512], F16, tag="sexp", bufs=12,
                                      name=f"se{i}_{n}")
                        nc.scalar.activation(
                            se[:], ps_s[:],
                            mybir.ActivationFunctionType.Exp,
                            bias=0.0, scale=float(SCALE))
                        if mid is not None:
                            nc.gpsimd.tensor_mul(se[:], se[:],
                                                 mask_sb[:, bass.ts(mid, 512)])
                        se_list.append((j, se))
                    return se_list

                def pv_pass(i2, se_list, hpair):
                    """PV + normalize for heads (2*hpair, 2*hpair+1)."""
                    njobs2 = len(se_list)
                    pso = [opp.tile([128, D + 1], F32, tag="oaug",
                                    name=f"pso{i2}_{2 * hpair + m}")
                           for m in range(2)]
                    for n, (j, se_t) in enumerate(se_list):
                        for m in range(2):
                            h = 2 * hpair + m
                            nc.tensor.matmul(
                                pso[m][:], se_t[:, bass.ds(128 * h, 128)],
                                vaug_sb[j][:], start=(n == 0),
                                stop=(n == njobs2 - 1))
                    ans = []
                    for m in range(2):
                        h = 2 * hpair + m
                        recip = asmall.tile([128, 1], F32, tag="recip",
                                            name=f"rc{i2}_{h}")
                        nc.vector.reciprocal(recip[:], pso[m][:, D : D + 1])
                        a_n = asp.tile([128, 128], F16, tag="anorm",
                                       bufs=10, name=f"an{i2}_{h}")
                        nc.scalar.mul(a_n[:], pso[m][:, 0:D], recip[:, 0:1])
                        ans.append(a_n)
                    return ans

                # software pipeline over token tiles: every PE instruction
                # in iteration i depends only on >= 1-iteration-old work,
                # so the in-order PE queue never waits on ACT/DVE chains.
                seL, anL, atL = {}, {}, {}
                for i in range(NT + 3):
                    if i < NT:
                        seL[i] = scores_block(i)
                    ans0 = None
                    if 0 <= i - 1 < NT:
                        ans0 = pv_pass(i - 1, seL[i - 1], 0)
                    if 0 <= i - 2 < NT:
                        atL[i - 2] = at_split(i - 2, anL.pop(i - 2))
                    if 0 <= i - 1 < NT:
                        anL[i - 1] = ans0 + pv_pass(i - 1, seL.pop(i - 1), 1)
                    if 0 <= i - 3 < NT:
                        oproj(i - 3, atL.pop(i - 3))

    nc.compile()
    return nc


def _get_nc(cu_seqlens):
    key = np.asarray(cu_seqlens).tobytes()
    if key not in _cache:
        jobs, masks_np, cos_t, sin_s, ident = _host_prep(cu_seqlens)
        nc = _build(jobs, masks_np.shape[0])
        _cache[key] = (nc, masks_np, cos_t, sin_s, ident)
    return _cache[key]


def kernel(hidden_states, Wq, Wk, Wv, Wo, cu_seqlens):
    hidden_states = np.asarray(hidden_states)
    Wq, Wk, Wv, Wo = (np.asarray(a) for a in (Wq, Wk, Wv, Wo))
    cu_seqlens = np.asarray(cu_seqlens)
    nc, masks_np, cos_t, sin_s, ident = _get_nc(cu_seqlens)

    ht = np.ascontiguousarray(hidden_states.T)  # [4096, 2048] f32
    ht8, dht8 = _split8(ht)
    # tile: [NSTRIP, NKT//4, 128, 2048] each - 4 hidden k-tiles side by side
    def _tile_ht(a):
        return np.ascontiguousarray(
            a.reshape(NKT // 4, 4, 128, NSTRIP, 512).transpose(3, 0, 2, 1, 4)
        ).reshape(NSTRIP, NKT // 4, 128, 2048)
    sht = np.concatenate([_tile_ht(ht8), _tile_ht(dht8)],
                         axis=3)  # [NSTRIP, 8, 128, 4096] e4m3
    # merge chunk pairs: [NSTRIP, 4, 128, 8192]
    sht = np.ascontiguousarray(
        sht.reshape(NSTRIP, 4, 2, 128, 4096).transpose(0, 1, 3, 2, 4)
    ).reshape(NSTRIP, 4, 128, 8192)

    in_maps = []
    for c in range(N_CORES):
        def wtile(wc):  # [HID, ncols] -> lhsT tiles [ncols//128, 128, HID]
            ncols = wc.shape[1]
            return np.ascontiguousarray(
                wc.reshape(NKT, 128, ncols // 128, 128).transpose(2, 1, 0, 3)
            ).reshape(ncols // 128, 128, HID)

        wq8, dwq8 = _split8(Wq[:, QD * c : QD * (c + 1)], E4, E5)
        wk8, dwk8 = _split8(Wk[:, D * c : D * (c + 1)], E4, E5)
        wv8, dwv8 = _split8(Wv[:, D * c : D * (c + 1)], E4, E5)
        wo8, dwo8 = _split8(Wo[QD * c : QD * (c + 1), :], E4, E5)

        def wpack(arrs):
            """6 lhsT arrays [128, HID] -> [8, 128, 6*512] chunk-packed."""
            a = np.stack(arrs)  # [6, 128, HID]
            a = a.reshape(6, 128, 8, 512).transpose(2, 1, 0, 3)
            return np.ascontiguousarray(a).reshape(8, 128, 3072)

        wq8t, dwq8t = wtile(wq8), wtile(dwq8)
        wpk4 = wpack([wq8t[0], wq8t[1], wq8t[2], wq8t[3],
                      wtile(wk8)[0], wtile(wv8)[0]])
        wpk5 = wpack([dwq8t[0], dwq8t[1], dwq8t[2], dwq8t[3],
                      wtile(dwk8)[0], wtile(dwv8)[0]])

        # wo: [512, 4096] -> [128, ns(8), hp(2), m(2), 512] -> [2, 128, 8192]
        def wotile2(w):
            # w: [512, 4096]; result [128, ns(8), hp(2), m(2), 512]
            r = np.empty((128, 8, 2, 2, 512), w.dtype)
            for ns in range(8):
                for hp in range(2):
                    for m in range(2):
                        h = 2 * hp + m
                        r[:, ns, hp, m, :] = w[128 * h : 128 * (h + 1),
                                               512 * ns : 512 * (ns + 1)]
            flat = r.reshape(128, 16384)
            return np.ascontiguousarray(
                np.stack([flat[:, 0:8192], flat[:, 8192:16384]], axis=0))

        in_maps.append({
            "sht": sht,
            "wpk4": wpk4, "wpk5": wpk5,
            "wo8": wotile2(wo8), "dwo8": wotile2(dwo8),
            "cos_t": cos_t, "sin_s": sin_s, "ident": ident,
            "masks": np.ascontiguousarray(
                masks_np.transpose(1, 0, 2).reshape(128, -1)),
        })

    res = bass_utils.run_bass_kernel_spmd(nc, in_maps,
                                          core_ids=list(range(N_CORES)))
    out = res.results[0]["out"].astype(np.float32)
    for c in range(1, N_CORES):
        out += res.results[c]["out"].astype(np.float32)
    return out
